# revision 1
# baseline (speedup 1.0000x reference)
"""Trainium2 Bass kernel for batch-all triplet margin loss (N=512, D=128).

Math:
  dist[i,g] = ||x_i - x_g||  (the reference's +1e-6 inside the norm shifts
  d2 by ~3e-5 -- far below bf16 noise, so it is dropped)
  loss = mean over valid (i,j,g) of relu(dist[i,j] - dist[i,g] + margin)
  valid: labels[j]==labels[i], j != i, labels[g] != labels[i]

Device strategy (SPMD over 8 cores, 64 anchors each, rows duplicated x2 so
each main-loop pass covers two positive ordinals):
  - X is rounded to bf16 once on the host; all device matmuls are bf16
    (1 cycle/row on PE, half the DMA bytes), and r = ||x||^2 plus the
    positive-pair biases a[p,u] = ||x_i - x_j|| + margin are computed on
    the host in f64 FROM the rounded X, so the bias side carries no extra
    rounding at all (the O(N*jmax*D) bias prep is ~1.5% of the kernel's
    O(N^2 D + N^2 jmax) work; the distance matrix and the 7.7M-term
    triplet reduction stay on device).
  - negative-distance blocks build in PSUM as (-2 X_a X^T) + a low-rank
    augment matmul carrying r_g (hi/lo bf16 split) and a 2^20 same-class
    mask (rank-16, one row per class); r_i enters exactly via the
    per-partition fp32 bias of the sqrt activation.  Masked distances
    come out EXACTLY 1024 in bf16.
  - main loop, one instruction per ordinal-pair u:
      DVE: tensor_scalar op0=max/op1=add: acc_u = sum_g max(d_g, a_u)
           (bf16 SBUF operands -> 4x_2p DVE mode, ~165ns/pass); the host
           recovers sum_g relu(a_u-d_g) = acc_u - rowsum(d) via a rowsum
           column (a DVE max-pass with bias -1e9); masked columns add
           exactly 1024 to both sides and cancel.
      ACT: relu(-d + a_u) with accum_out (direct relu sums).
    Self/invalid ordinals get a = margin, so they contribute exactly 0.
  - pass-count: the main tile needs only ceil(m2/2) passes (m2 = second
    largest class size).  The largest class's remaining ranks are handled
    by ONE tail pass per core (ACT relu): every core holds a replica of
    the big class's anchors (dup x2 <= 128 slots) with its own d2 block,
    and core c's bias column carries rank pair (2*umax_m + 2c, +1).
  - acc [128, umax_m + 2]: main cols | tail col (ACT relu) | rowsum col.
    One output DMA; host reduces and divides by the exact triplet count.
"""

import numpy as np
import ml_dtypes

BF = ml_dtypes.bfloat16
N, D, C = 512, 128, 16
NCORES = 8
APC = N // NCORES  # 64 anchors per core
# 2^20 -> masked distances are exactly bf16 1024 (sqrt(2^20+d2) rounds to
# 1024 for d2 <= ~2000 since the bf16 step at 1024 is 8)
MASK = float(2.0**20)
PADBIAS = 1.0e9  # pad/rowsum slots get bias -PADBIAS

_CACHE = {}


def _build_program(umax, margin, act_us):
    """umax here is umax_m (main ordinal-pair count); +2 acc columns hold
    the tail pass and the rowsum."""
    import concourse.bacc as bacc
    import concourse.tile as tile
    from concourse import mybir

    fp32 = mybir.dt.float32
    bf16 = mybir.dt.bfloat16
    AF = mybir.ActivationFunctionType
    OP = mybir.AluOpType

    # pb columns: 0 r_i | 1 r47 | 2..2+umax main biases | 2+umax tail bias
    # | 3+umax rowsum bias (-1e9)
    pbw = 4 + umax

    nc = bacc.Bacc("TRN2", target_bir_lowering=False, debug=False)
    pa = nc.declare_dram_parameter("pa", [128, 640], bf16, isOutput=False)
    pa47 = nc.declare_dram_parameter("pa47", [128, 128], bf16, isOutput=False)
    pk2 = nc.declare_dram_parameter("pk2", [18, 1280], bf16, isOutput=False)
    pb = nc.declare_dram_parameter("pb", [128, pbw], fp32, isOutput=False)
    acc_out = nc.declare_dram_parameter(
        "acc", [128, umax + 2], fp32, isOutput=True
    )

    with tile.TileContext(nc) as tc:
        with (
            tc.tile_pool(name="io", bufs=1) as io,
            tc.tile_pool(name="work", bufs=1) as work,
            tc.tile_pool(name="psum", bufs=1, space="PSUM") as psum,
        ):
            t_pa = io.tile([128, 640], bf16)
            t_pa47 = io.tile([128, 128], bf16)
            t_pk2 = io.tile([18, 1280], bf16)
            t_pb = io.tile([128, pbw], fp32)
            # a dummy activation FIRST in program order makes the
            # act-table-load pass insert its LoadActFuncSet at the top of
            # the block, so the 1283ns load overlaps the input DMAs
            t_dummy = work.tile([128, 1], fp32, tag="dummy")
            nc.gpsimd.memset(t_dummy[:], 1.0)
            t_dummy2 = work.tile([128, 1], bf16, tag="dummy2")
            nc.scalar.activation(t_dummy2[:], t_dummy[:], AF.Sqrt)
            # ACT queue stays free of DMA issues so its sequencer reaches
            # the act-table load immediately
            nc.sync.dma_start(t_pa[:], pa[:])
            nc.sync.dma_start(t_pk2[:], pk2[:])
            nc.sync.dma_start(t_pb[:], pb[:])
            nc.gpsimd.dma_start(t_pa47[:], pa47[:])

            xia = t_pa[:, 0:128]           # X_Idup^T [d, 128]
            xga = t_pa[:, 128:640]         # -2 X^T [d, 512]
            x47 = t_pa47[:, 0:128]         # X_47 dup [d, 128]

            # ---- big d2 (main): [slot(128), g(512)] ----
            # high_priority: the scheduler must not slot the tail matmuls
            # between the main product and augment -- bneg gates the loop
            p_d2 = psum.tile([128, N], fp32, tag="d2")
            with tc.high_priority():
                nc.tensor.matmul(p_d2[:], xia, xga, start=True, stop=False)
                nc.tensor.matmul(
                    p_d2[:], t_pk2[0:18, 0:128], t_pk2[0:18, 128:640],
                    start=False, stop=True,
                )
            # ---- tail: big d2 block [47-slot(128), g(512)] ----
            p_d247 = psum.tile([128, N], fp32, tag="d247")
            nc.tensor.matmul(p_d247[:], x47, xga, start=True, stop=False)
            nc.tensor.matmul(
                p_d247[:], t_pk2[0:3, 640:768], t_pk2[0:3, 768:1280],
                start=False, stop=True,
            )

            # ---- sqrts (ACT) ----
            t_bneg = work.tile([128, N], bf16, tag="bneg")
            nc.scalar.activation(t_bneg[:], p_d2[:], AF.Sqrt, bias=t_pb[:, 0:1])
            t_bneg47 = work.tile([128, N], bf16, tag="bneg47")
            nc.scalar.activation(
                t_bneg47[:], p_d247[:], AF.Sqrt, bias=t_pb[:, 1:2]
            )

            # ---- main loop ----
            t_acc = work.tile([128, umax + 2], fp32, tag="acc")
            t_trash_d = [
                work.tile([128, N], bf16, name=f"trd{i}", tag=f"trd{i}")
                for i in range(2)
            ]
            t_trash_a = [
                work.tile([128, N], bf16, name=f"tra{i}", tag=f"tra{i}")
                for i in range(2)
            ]
            # rowsum(d) column for the max-sum correction: a DVE max-pass
            # with bias -1e9 (max(d, -1e9) = d, so the accum is rowsum)
            nc.vector.tensor_scalar(
                t_trash_d[1][:],
                t_bneg[:],
                t_pb[:, 3 + umax : 4 + umax],
                None,
                op0=OP.max,
                op1=OP.add,
                accum_out=t_acc[:, umax + 1 : umax + 2],
            )
            nd = na = 0
            for u in range(umax):
                if u in act_us:
                    nc.scalar.activation(
                        t_trash_a[na % 2][:],
                        t_bneg[:],
                        AF.Relu,
                        bias=t_pb[:, 2 + u : 3 + u],
                        scale=-1.0,
                        accum_out=t_acc[:, u : u + 1],
                    )
                    na += 1
                else:
                    nc.vector.tensor_scalar(
                        t_trash_d[nd % 2][:],
                        t_bneg[:],
                        t_pb[:, 2 + u : 3 + u],
                        None,
                        op0=OP.max,
                        op1=OP.add,
                        accum_out=t_acc[:, u : u + 1],
                    )
                    nd += 1
            # tail pass: big-class ranks for this core's rank pair (ACT)
            nc.scalar.activation(
                t_trash_a[na % 2][:],
                t_bneg47[:],
                AF.Relu,
                bias=t_pb[:, 2 + umax : 3 + umax],
                scale=-1.0,
                accum_out=t_acc[:, umax : umax + 1],
            )

            nc.sync.dma_start(acc_out[:], t_acc[:])

    nc.finalize()
    return nc


def plan(outputs, labels, margin, n_act=2):
    """Build (nc, in_maps, umax_m, count); shared by kernel() and test."""
    X64 = np.asarray(outputs, dtype=np.float64)
    lab = np.asarray(labels).astype(np.int64).reshape(-1)
    margin = float(margin)
    assert X64.shape == (N, D) and lab.shape == (N,)

    Xb = X64.astype(BF)                      # round once
    Xw = Xb.astype(np.float64)               # exact value of the rounding
    r = (Xw * Xw).sum(1)                     # row norms of the rounded X

    nclass = max(C, int(lab.max()) + 1)
    m = np.bincount(lab, minlength=nclass)
    jmax = int(m.max())
    cbig = int(m.argmax())
    m2 = int(np.sort(m)[-2])
    umax = (m2 + 1) // 2                     # main ordinal pairs
    tp = max(0, (jmax - 2 * umax + 1) // 2)  # tail rank pairs
    assert tp <= NCORES, (jmax, m2, tp)
    I47 = np.flatnonzero(lab == cbig)
    n47 = len(I47)
    assert n47 <= APC
    count = float(sum(int(mc) * (int(mc) - 1) * (N - int(mc)) for mc in m))

    rank = np.zeros(N, dtype=np.int64)
    cnt = {}
    for j in range(N):
        c = int(lab[j])
        rank[j] = cnt.get(c, 0)
        cnt[c] = cnt.get(c, 0) + 1
    # members[c][rho] = index of the class-c member with rank rho
    members = {c: np.flatnonzero(lab == c)[np.argsort(rank[lab == c])]
               for c in range(nclass) if m[c]}

    n_act = max(0, min(n_act, umax))
    act_us = frozenset(range(umax - n_act, umax))
    global _LAST_ACT_US
    _LAST_ACT_US = act_us

    key = (umax, margin, act_us)
    if key not in _CACHE:
        _CACHE[key] = _build_program(umax, margin, act_us)
    nc = _CACHE[key]

    r_hi = r.astype(BF)
    r_lo = (r - r_hi.astype(np.float64)).astype(BF)
    onehot = lab[None, :] == np.arange(nclass)[:C, None]  # [16, 512]

    def pos_dist(i, j):
        dd = Xw[i] - Xw[j]
        return np.sqrt((dd * dd).sum())

    # bias for a slot list: slot p covers anchor anchors[p] at rank
    # 2u + parity[p]; self and missing ordinals contribute a = margin,
    # which is below every unmasked distance -> relu 0
    def bias_matrix(anchor_ids, parity, ucount, base_rank=0):
        nslot = len(anchor_ids)
        A = np.full((nslot, ucount), margin, dtype=np.float64)
        for p in range(nslot):
            i = anchor_ids[p]
            if i < 0:
                A[p, :] = -PADBIAS
                continue
            mem = members[int(lab[i])]
            for u in range(ucount):
                rho = base_rank + 2 * u + parity[p]
                if rho < len(mem) and mem[rho] != i:
                    A[p, u] = pos_dist(i, mem[rho]) + margin
        return A

    # X47 dup block [d, 128]
    X47blk = np.zeros((D, 128), dtype=BF)
    X47blk[:, 0:n47] = Xb[I47].T
    X47blk[:, 64 : 64 + n47] = Xb[I47].T
    r47 = np.zeros(128)
    r47[0:n47] = r[I47]
    r47[64 : 64 + n47] = r[I47]
    slot47 = np.full(128, -1, dtype=np.int64)
    slot47[0:n47] = I47
    slot47[64 : 64 + n47] = I47
    par47 = np.zeros(128, dtype=np.int64)
    par47[64:128] = 1

    pbw = 4 + umax
    in_maps = []
    for c in range(NCORES):
        I = np.arange(c * APC, (c + 1) * APC)
        Idup = np.concatenate([I, I])
        pardup = np.concatenate([np.zeros(APC, np.int64), np.ones(APC, np.int64)])

        PA = np.empty((128, 640), dtype=BF)
        PA[:, 0:128] = Xb[Idup].T
        PA[:, 128:640] = (-2.0 * Xw).astype(BF).T  # exact: power-of-2 scale

        PK2 = np.zeros((18, 1280), dtype=BF)
        # main big augment: lhsT [18,128] at 0:128, rhs [18,512] at 128:640
        PK2[0, 0:128] = 1.0
        PK2[1, 0:128] = 1.0
        PK2[2:18, 0:128] = np.where(onehot[:, Idup], MASK, 0.0)
        PK2[0, 128:640] = r_hi
        PK2[1, 128:640] = r_lo
        PK2[2:18, 128:640] = onehot.astype(np.float64)
        # tail big augment: lhsT [3,128] at 640:768, rhs [3,512] at 768:1280
        PK2[0, 640:768] = 1.0
        PK2[1, 640:768] = 1.0
        PK2[2, 640:768] = MASK
        PK2[0, 768:1280] = r_hi
        PK2[1, 768:1280] = r_lo
        PK2[2, 768:1280] = onehot[cbig].astype(np.float64)

        PB = np.zeros((128, pbw), dtype=np.float32)
        PB[:, 0] = r[Idup]
        PB[:, 1] = r47
        PB[:, 2 : 2 + umax] = bias_matrix(Idup, pardup, umax)
        if c < tp:
            PB[:, 2 + umax] = bias_matrix(
                slot47, par47, 1, base_rank=2 * umax + 2 * c
            )[:, 0]
        else:
            PB[:, 2 + umax] = -PADBIAS
        PB[:, 3 + umax] = -PADBIAS

        in_maps.append({"pa": PA, "pa47": np.ascontiguousarray(X47blk), "pk2": PK2, "pb": PB})

    return nc, in_maps, umax, count


_LAST_ACT_US = frozenset()


def reduce_results(results, umax, count):
    # ACT cols (act_us and the tail col umax) hold relu sums directly; DVE
    # cols hold sum_g max(d_g, a_u) and need the rowsum col (umax+1)
    # subtracted.
    total = 0.0
    for c in range(NCORES):
        acc = results[c]["acc"].astype(np.float64)  # [128, umax+2]
        rs = acc[:, umax + 1].sum()
        for u in range(umax):
            cs = acc[:, u].sum()
            total += cs if u in _LAST_ACT_US else cs - rs
        total += acc[:, umax].sum()  # tail col
    return np.float32(total / count)


def kernel(outputs, labels, margin):
    from concourse.bass_utils import run_bass_kernel_spmd

    nc, in_maps, umax, count = plan(outputs, labels, margin)
    res = run_bass_kernel_spmd(nc, in_maps, list(range(NCORES)))
    loss = reduce_results(res.results, umax, count)
    return (loss, 0.0, 0.0, 0.0)

